# revision 2
# baseline (speedup 1.0000x reference)
"""Multi-head attention (B=64, N=512, C=1024, H=16) on 8 TRN2 NeuronCores.

Sharding: pure data-parallel over batch — each core computes 8 batches
end-to-end (no collectives). Returns (out [64,512,512] f32,
probs [1024,512,512] f32) matching the reference.

Per-core pipeline (all matmuls bf16 into f32 PSUM):
  x -> xT (PE transpose)  -> QT/KT (c-on-partition) and V (natural)
  S = QK^T per head, row-packed K=64 matmul pairs into [128,1024] PSUM
  E = exp(S/8) via ACT (bf16 out, f32 rowsums via accum_out)
  P = E * (1/rowsum)  (DVE tensor_scalar, bf16)  -> probs out (SWDGE cast DMA)
  P.T via xbar DMA-transpose (bf16, SBUF->SBUF)
  ctxT = V^T-layout PV matmul, col-packed pairs -> out = ctxT @ WoT + bo
"""
import numpy as np

B_TOTAL, N, C, H = 64, 512, 1024, 16
NCORES = 8
BS = B_TOTAL // NCORES       # 8 batches per core
NT = N // 128                # 4 sequence tiles
CT = C // 128                # 8 channel tiles
HPAIRS = H // 2              # 8 head pairs (one per c-tile of QT/KT)
SCALE = 0.125                # 1/sqrt(64)


def build_attention_nc(nbatch=BS, repeats=1):
    """Build the per-core Bass graph. repeats>1 wraps the whole body in a
    For_i loop (identical re-execution) for differential wall-clock timing."""
    import concourse.mybir as mybir
    import concourse.tile as tile
    from concourse import bacc
    from concourse.masks import make_identity

    F32 = mybir.dt.float32
    BF16 = mybir.dt.bfloat16
    EXP = mybir.ActivationFunctionType.Exp
    IDENT = mybir.ActivationFunctionType.Identity

    nc = bacc.Bacc("TRN2", target_bir_lowering=False, debug=False)
    x_ext = nc.dram_tensor("x", [nbatch, N, C], F32, kind="ExternalInput").ap()
    wq_ext = nc.dram_tensor("Wq", [C, C], F32, kind="ExternalInput").ap()
    bq_ext = nc.dram_tensor("bq", [C], F32, kind="ExternalInput").ap()
    wk_ext = nc.dram_tensor("Wk", [C, C], F32, kind="ExternalInput").ap()
    bk_ext = nc.dram_tensor("bk", [C], F32, kind="ExternalInput").ap()
    wv_ext = nc.dram_tensor("Wv", [C, C], F32, kind="ExternalInput").ap()
    bv_ext = nc.dram_tensor("bv", [C], F32, kind="ExternalInput").ap()
    wo_ext = nc.dram_tensor("Wo", [N, C], F32, kind="ExternalInput").ap()
    bo_ext = nc.dram_tensor("bo", [N], F32, kind="ExternalInput").ap()
    out_ext = nc.dram_tensor("out", [nbatch, N, N], F32, kind="ExternalOutput").ap()
    probs_ext = nc.dram_tensor("probs", [nbatch * H, N, N], F32,
                               kind="ExternalOutput").ap()

    with tile.TileContext(nc) as tc:
        with (
            tc.tile_pool(name="const", bufs=1) as constp,
            tc.tile_pool(name="wts", bufs=1) as wts,
            tc.tile_pool(name="wnat", bufs=9) as wnatp,
            tc.tile_pool(name="work", bufs=1) as work,
            tc.tile_pool(name="ps_s", bufs=2, space="PSUM") as ps_s,
            tc.tile_pool(name="ps_m", bufs=3, space="PSUM") as ps_m,
            tc.tile_pool(name="ps_t", bufs=1, space="PSUM") as ps_t,
        ):
            ident = constp.tile([128, 128], BF16, tag="ident")
            make_identity(nc, ident)
            ones = constp.tile([1, 128], BF16, tag="ones")
            nc.vector.memset(ones, 1.0)
            # bq/bk as [128, CT] (column ct = bias slice for c-tile ct)
            bq_sb = constp.tile([128, CT], F32, tag="bq")
            nc.sync.dma_start(out=bq_sb, in_=bq_ext.rearrange("(t p) -> p t", p=128))
            bk_sb = constp.tile([128, CT], F32, tag="bk")
            nc.sync.dma_start(out=bk_sb, in_=bk_ext.rearrange("(t p) -> p t", p=128))
            bv_row = constp.tile([1, C], BF16, tag="bv")
            nc.gpsimd.dma_start(out=bv_row, in_=bv_ext[None, :])
            bo_row = constp.tile([1, N], BF16, tag="bo")
            nc.gpsimd.dma_start(out=bo_row, in_=bo_ext[None, :])

            # ---- weight prep: W[c,k] -> WT tiles [k-part, c-free] (bf16)
            def transpose_weight(w_ext, n_ctiles, out_free, out_tag):
                # w_ext: [n_ctiles*128, CT*128]; returns list over kc of
                # [128, out_free] bf16 tiles (out_free = n_ctiles*128)
                nat = []
                for ctl in range(n_ctiles):
                    wn = wnatp.tile([128, C], BF16, tag="wnat")
                    nc.gpsimd.dma_start(
                        out=wn, in_=w_ext[ctl * 128:(ctl + 1) * 128, :])
                    nat.append(wn)
                outs = []
                for kc in range(CT):
                    ptr = ps_t.tile([128, out_free], BF16, tag="tr")
                    for ctl in range(n_ctiles):
                        nc.tensor.transpose(
                            ptr[:, ctl * 128:(ctl + 1) * 128],
                            nat[ctl][:, kc * 128:(kc + 1) * 128], ident)
                    wt = wts.tile([128, out_free], BF16, tag=f"{out_tag}{kc}")
                    nc.vector.tensor_copy(wt, ptr)
                    outs.append(wt)
                return outs

            wqt = transpose_weight(wq_ext, CT, C, "wqt")
            wkt = transpose_weight(wk_ext, CT, C, "wkt")
            wvt = transpose_weight(wv_ext, CT, C, "wvt")
            wot = transpose_weight(wo_ext, NT, N, "wot")

            def body(_iv):
                for b in range(nbatch):
                    # ---- xT
                    xb = []
                    for mt in range(NT):
                        xbt = work.tile([128, C], BF16, tag="xb", bufs=5)
                        nc.gpsimd.dma_start(
                            out=xbt, in_=x_ext[b, mt * 128:(mt + 1) * 128, :])
                        xb.append(xbt)
                    xt = []
                    for kc in range(CT):
                        ptr = ps_t.tile([128, N], BF16, tag="tr")
                        for mt in range(NT):
                            nc.tensor.transpose(
                                ptr[:, mt * 128:(mt + 1) * 128],
                                xb[mt][:, kc * 128:(kc + 1) * 128], ident)
                        xtt = work.tile([128, N], BF16, tag="xt", bufs=9)
                        nc.vector.tensor_copy(xtt, ptr)
                        xt.append(xtt)

                    # ---- QT / KT  [c-part, m-free]
                    qt, kt = [], []
                    for dst, wt, bias_sb in ((qt, wqt, bq_sb), (kt, wkt, bk_sb)):
                        for ct in range(CT):
                            pq = ps_m.tile([128, N], F32, tag="pm")
                            for kc in range(CT):
                                nc.tensor.matmul(
                                    pq, wt[kc][:, ct * 128:(ct + 1) * 128],
                                    xt[kc], start=(kc == 0), stop=(kc == CT - 1))
                            qtt = work.tile([128, N], BF16, bufs=9,
                                            tag="qt" if dst is qt else "kt")
                            nc.scalar.activation(qtt, pq, IDENT,
                                                 bias=bias_sb[:, ct:ct + 1])
                            dst.append(qtt)

                    # ---- V natural [j-part, c-free]
                    vv = []
                    for mt in range(NT):
                        vvt = work.tile([128, C], BF16, tag="vv", bufs=5)
                        for ch in range(2):
                            pv = ps_m.tile([128, N], F32, tag="pm")
                            nc.tensor.matmul(pv, ones,
                                             bv_row[0:1, ch * 512:(ch + 1) * 512],
                                             start=True, stop=False)
                            for kc in range(CT):
                                nc.tensor.matmul(
                                    pv, xt[kc][:, mt * 128:(mt + 1) * 128],
                                    wvt[kc][:, ch * 512:(ch + 1) * 512],
                                    start=False, stop=(kc == CT - 1))
                            nc.vector.tensor_copy(
                                vvt[:, ch * 512:(ch + 1) * 512], pv)
                        vv.append(vvt)

                    # ---- attention per head pair
                    ctxt = []
                    for hp in range(HPAIRS):
                        sums = work.tile([128, 8], F32, tag="sums", bufs=3)
                        e_t = {}  # (head, half) -> [128, 1024] bf16
                        for half in range(2):
                            sA = ps_s.tile([128, 1024], F32, tag="ps")
                            sB = ps_s.tile([128, 1024], F32, tag="ps")
                            for q in range(2):
                                it = 2 * half + q
                                nc.tensor.matmul(
                                    sA[:, q * 512:(q + 1) * 512],
                                    qt[hp][0:64, it * 128:(it + 1) * 128],
                                    kt[hp][0:64, :], start=True, stop=True,
                                    tile_position=(0, 0))
                                nc.tensor.matmul(
                                    sB[:, q * 512:(q + 1) * 512],
                                    qt[hp][64:128, it * 128:(it + 1) * 128],
                                    kt[hp][64:128, :], start=True, stop=True,
                                    tile_position=(64, 0))
                            eA = work.tile([128, 1024], BF16, tag="e", bufs=6)
                            eB = work.tile([128, 1024], BF16, tag="e", bufs=6)
                            for q in range(2):
                                it = 2 * half + q
                                nc.scalar.activation(
                                    eA[:, q * 512:(q + 1) * 512],
                                    sA[:, q * 512:(q + 1) * 512], EXP,
                                    scale=SCALE,
                                    accum_out=sums[:, it:it + 1])
                                nc.scalar.activation(
                                    eB[:, q * 512:(q + 1) * 512],
                                    sB[:, q * 512:(q + 1) * 512], EXP,
                                    scale=SCALE,
                                    accum_out=sums[:, 4 + it:5 + it])
                            e_t[(0, half)] = eA
                            e_t[(1, half)] = eB
                        rec = work.tile([128, 8], F32, tag="rec", bufs=3)
                        nc.vector.reciprocal(rec, sums)

                        pts = []
                        for head in range(2):
                            p_t = work.tile([128, 4, 512], BF16, tag="p", bufs=3)
                            for it in range(4):
                                nc.vector.tensor_scalar_mul(
                                    p_t[:, it, :],
                                    e_t[(head, it // 2)][:, (it % 2) * 512:
                                                         (it % 2 + 1) * 512],
                                    rec[:, head * 4 + it:head * 4 + it + 1])
                            bh = b * H + hp * 2 + head
                            nc.gpsimd.dma_start(
                                out=probs_ext[bh].rearrange(
                                    "(it p) j -> p it j", p=128),
                                in_=p_t)
                            pt_t = work.tile([128, 4, 4, 128], BF16, tag="pt", bufs=4)
                            for it in range(4):
                                nc.sync.dma_start(
                                    out=pt_t[:, :, it, :],
                                    in_=p_t[:, it, :], transpose=True)
                            pts.append(pt_t)

                        pctx = ps_m.tile([128, N], F32, tag="pm")
                        for jt in range(NT):
                            nc.tensor.matmul(
                                pctx[0:64, :],
                                vv[jt][:, (2 * hp) * 64:(2 * hp + 1) * 64],
                                pts[0][:, jt], start=(jt == 0),
                                stop=(jt == NT - 1), tile_position=(0, 0))
                            nc.tensor.matmul(
                                pctx[64:128, :],
                                vv[jt][:, (2 * hp + 1) * 64:(2 * hp + 2) * 64],
                                pts[1][:, jt], start=(jt == 0),
                                stop=(jt == NT - 1), tile_position=(0, 64))
                        ctt = work.tile([128, N], BF16, tag="ctxt", bufs=9)
                        nc.vector.tensor_copy(ctt, pctx)
                        ctxt.append(ctt)

                    # ---- out projection
                    for mt in range(NT):
                        po = ps_m.tile([128, N], F32, tag="pm")
                        nc.tensor.matmul(po, ones, bo_row,
                                         start=True, stop=False)
                        for ct in range(CT):
                            nc.tensor.matmul(
                                po, ctxt[ct][:, mt * 128:(mt + 1) * 128],
                                wot[ct], start=False, stop=(ct == CT - 1))
                        ott = work.tile([128, N], F32, tag="outt", bufs=3)
                        nc.vector.tensor_copy(ott, po)
                        nc.sync.dma_start(
                            out=out_ext[b, mt * 128:(mt + 1) * 128, :], in_=ott)

            if repeats == 1:
                body(0)
            else:
                with tc.For_i(0, repeats, 1) as iv:
                    body(iv)

    nc.compile()
    return nc


_CACHED = {}


def _get_nc():
    if "nc" not in _CACHED:
        _CACHED["nc"] = build_attention_nc()
    return _CACHED["nc"]


def kernel(**inputs):
    from concourse.bass_utils import run_bass_kernel_spmd

    nc = _get_nc()
    x = np.ascontiguousarray(np.asarray(inputs["x"], dtype=np.float32))
    shared = {k: np.ascontiguousarray(np.asarray(inputs[k], dtype=np.float32))
              for k in ("Wq", "bq", "Wk", "bk", "Wv", "bv", "Wo", "bo")}
    in_maps = [dict(x=x[c * BS:(c + 1) * BS], **shared) for c in range(NCORES)]
    res = run_bass_kernel_spmd(nc, in_maps, core_ids=list(range(NCORES)))
    out = np.concatenate([res.results[c]["out"] for c in range(NCORES)], axis=0)
    probs = np.concatenate([res.results[c]["probs"] for c in range(NCORES)],
                           axis=0)
    return out, probs


# revision 5
# speedup vs baseline: 1.5235x; 1.5235x over previous
"""Multi-head attention (B=64, N=512, C=1024, H=16) on 8 TRN2 NeuronCores.

Sharding: pure data-parallel over batch — each core computes 8 batches
end-to-end (no collectives). Returns (out [64,512,512] f32,
probs [1024,512,512] f32) matching the reference.

Per-core pipeline (all matmuls bf16 into f32 PSUM):
  x -> xT (PE transpose)  -> QT/KT (c-on-partition) and V (natural)
  S = QK^T per head, row-packed K=64 matmul pairs into [128,1024] PSUM
  E = exp(S/8) via ACT (bf16 out, f32 rowsums via accum_out)
  P = E * (1/rowsum)  (DVE tensor_scalar, bf16)  -> probs out (SWDGE cast DMA)
  P.T via xbar DMA-transpose (bf16, SBUF->SBUF)
  ctxT = V^T-layout PV matmul, col-packed pairs -> out = ctxT @ WoT + bo
"""
import numpy as np

B_TOTAL, N, C, H = 64, 512, 1024, 16
NCORES = 8
BS = B_TOTAL // NCORES       # 8 batches per core
NT = N // 128                # 4 sequence tiles
CT = C // 128                # 8 channel tiles
HPAIRS = H // 2              # 8 head pairs (one per c-tile of QT/KT)
SCALE = 0.125                # 1/sqrt(64)


def build_attention_nc(nbatch=BS, repeats=1, ablate=()):
    """Build the per-core Bass graph. repeats>1 wraps the whole body in a
    For_i loop (identical re-execution) for differential wall-clock timing."""
    import concourse.mybir as mybir
    import concourse.tile as tile
    from concourse import bacc
    from concourse.masks import make_identity

    F32 = mybir.dt.float32
    BF16 = mybir.dt.bfloat16
    EXP = mybir.ActivationFunctionType.Exp
    IDENT = mybir.ActivationFunctionType.Identity

    nc = bacc.Bacc("TRN2", target_bir_lowering=False, debug=False)
    x_ext = nc.dram_tensor("x", [nbatch, N, C], F32, kind="ExternalInput").ap()
    wq_ext = nc.dram_tensor("Wq", [C, C], F32, kind="ExternalInput").ap()
    bq_ext = nc.dram_tensor("bq", [C], F32, kind="ExternalInput").ap()
    wk_ext = nc.dram_tensor("Wk", [C, C], F32, kind="ExternalInput").ap()
    bk_ext = nc.dram_tensor("bk", [C], F32, kind="ExternalInput").ap()
    wv_ext = nc.dram_tensor("Wv", [C, C], F32, kind="ExternalInput").ap()
    bv_ext = nc.dram_tensor("bv", [C], F32, kind="ExternalInput").ap()
    wo_ext = nc.dram_tensor("Wo", [N, C], F32, kind="ExternalInput").ap()
    bo_ext = nc.dram_tensor("bo", [N], F32, kind="ExternalInput").ap()
    out_ext = nc.dram_tensor("out", [nbatch, N, N], F32, kind="ExternalOutput").ap()
    probs_ext = nc.dram_tensor("probs", [nbatch * H, N, N], F32,
                               kind="ExternalOutput").ap()

    with tile.TileContext(nc) as tc:
        with (
            tc.tile_pool(name="const", bufs=1) as constp,
            tc.tile_pool(name="wts", bufs=1) as wts,
            tc.tile_pool(name="wnat", bufs=9) as wnatp,
            tc.tile_pool(name="work", bufs=1) as work,
            tc.tile_pool(name="ps_s", bufs=4, space="PSUM") as ps_s,
            tc.tile_pool(name="ps_m", bufs=2, space="PSUM") as ps_m,
            tc.tile_pool(name="ps_r", bufs=1, space="PSUM") as ps_r,
            tc.tile_pool(name="ps_t", bufs=1, space="PSUM") as ps_t,
        ):
            ident = constp.tile([128, 128], BF16, tag="ident")
            make_identity(nc, ident)
            identf32 = constp.tile([128, 128], F32, tag="identf32")
            make_identity(nc, identf32)
            ones = constp.tile([1, 128], BF16, tag="ones")
            nc.vector.memset(ones, 1.0)
            # bq/bk as [128, CT] (column ct = bias slice for c-tile ct)
            bq_sb = constp.tile([128, CT], F32, tag="bq")
            nc.sync.dma_start(out=bq_sb, in_=bq_ext.rearrange("(t p) -> p t", p=128))
            bk_sb = constp.tile([128, CT], F32, tag="bk")
            nc.sync.dma_start(out=bk_sb, in_=bk_ext.rearrange("(t p) -> p t", p=128))
            bv_row = constp.tile([1, C], BF16, tag="bv")
            nc.gpsimd.dma_start(out=bv_row, in_=bv_ext[None, :])
            bo_row = constp.tile([1, N], BF16, tag="bo")
            nc.gpsimd.dma_start(out=bo_row, in_=bo_ext[None, :])

            # ---- weight prep: W[c,k] -> WT tiles [k-part, c-free] (bf16)
            def transpose_weight(w_ext, n_ctiles, out_free, out_tag):
                # w_ext: [n_ctiles*128, CT*128]; returns list over kc of
                # [128, out_free] bf16 tiles (out_free = n_ctiles*128)
                nat = []
                for ctl in range(n_ctiles):
                    wn = wnatp.tile([128, C], BF16, tag="wnat")
                    nc.gpsimd.dma_start(
                        out=wn, in_=w_ext[ctl * 128:(ctl + 1) * 128, :])
                    nat.append(wn)
                outs = []
                for kc in range(CT):
                    ptr = ps_t.tile([128, out_free], BF16, tag="tr")
                    for ctl in range(n_ctiles):
                        nc.tensor.transpose(
                            ptr[:, ctl * 128:(ctl + 1) * 128],
                            nat[ctl][:, kc * 128:(kc + 1) * 128], ident)
                    wt = wts.tile([128, out_free], BF16, tag=f"{out_tag}{kc}")
                    nc.vector.tensor_copy(wt, ptr)
                    outs.append(wt)
                return outs

            wqt = transpose_weight(wq_ext, CT, C, "wqt")
            wkt = transpose_weight(wk_ext, CT, C, "wkt")
            wvt = transpose_weight(wv_ext, CT, C, "wvt")
            wot = transpose_weight(wo_ext, NT, N, "wot")

            def body(_iv):
                for b in range(nbatch):
                    # ---- xT
                    xb = []
                    for mt in range(NT):
                        xbt = work.tile([128, C], BF16, tag="xb", bufs=5)
                        nc.gpsimd.dma_start(
                            out=xbt, in_=x_ext[b, mt * 128:(mt + 1) * 128, :])
                        xb.append(xbt)
                    xt = []
                    for kc in range(CT):
                        ptr = ps_t.tile([128, N], BF16, tag="tr")
                        for mt in range(NT):
                            nc.tensor.transpose(
                                ptr[:, mt * 128:(mt + 1) * 128],
                                xb[mt][:, kc * 128:(kc + 1) * 128], ident)
                        xtt = work.tile([128, N], BF16, tag="xt", bufs=9)
                        nc.vector.tensor_copy(xtt, ptr)
                        xt.append(xtt)

                    # ---- QT / KT  [c-part, m-free]
                    qt, kt = [], []
                    for dst, wt, bias_sb in ((qt, wqt, bq_sb), (kt, wkt, bk_sb)):
                        for ct in range(CT):
                            pq = ps_m.tile([128, N], F32, tag="pm")
                            for kc in range(CT):
                                nc.tensor.matmul(
                                    pq, wt[kc][:, ct * 128:(ct + 1) * 128],
                                    xt[kc], start=(kc == 0), stop=(kc == CT - 1))
                            qtt = work.tile([128, N], BF16, bufs=9,
                                            tag="qt" if dst is qt else "kt")
                            nc.scalar.activation(qtt, pq, IDENT,
                                                 bias=bias_sb[:, ct:ct + 1])
                            dst.append(qtt)

                    # ---- V natural [j-part, c-free]
                    vv = []
                    for mt in range(NT):
                        vvt = work.tile([128, C], BF16, tag="vv", bufs=5)
                        for ch in range(2):
                            pv = ps_m.tile([128, N], F32, tag="pm")
                            nc.tensor.matmul(pv, ones,
                                             bv_row[0:1, ch * 512:(ch + 1) * 512],
                                             start=True, stop=False)
                            for kc in range(CT):
                                nc.tensor.matmul(
                                    pv, xt[kc][:, mt * 128:(mt + 1) * 128],
                                    wvt[kc][:, ch * 512:(ch + 1) * 512],
                                    start=False, stop=(kc == CT - 1))
                            nc.vector.tensor_copy(
                                vvt[:, ch * 512:(ch + 1) * 512], pv)
                        vv.append(vvt)

                    # ---- attention per head pair
                    ctxt = []
                    for hp in range(HPAIRS):
                        sums = work.tile([128, 8], F32, tag="sums", bufs=3)
                        e_t = {}
                        for half in range(2):
                            eA = work.tile([128, 1024], BF16, tag="e", bufs=6)
                            eB = work.tile([128, 1024], BF16, tag="e", bufs=6)
                            for q in range(2):
                                it = 2 * half + q
                                sA = ps_s.tile([128, 512], F32, tag="ps")
                                sB = ps_s.tile([128, 512], F32, tag="ps")
                                nc.tensor.matmul(
                                    sA, qt[hp][0:64, it * 128:(it + 1) * 128],
                                    kt[hp][0:64, :], start=True, stop=True,
                                    tile_position=(0, 0))
                                nc.tensor.matmul(
                                    sB, qt[hp][64:128, it * 128:(it + 1) * 128],
                                    kt[hp][64:128, :], start=True, stop=True,
                                    tile_position=(64, 0))
                                if "exp" not in ablate:
                                    nc.scalar.activation(
                                        eA[:, q * 512:(q + 1) * 512], sA,
                                        EXP, scale=SCALE,
                                        accum_out=sums[:, it:it + 1])
                                    nc.scalar.activation(
                                        eB[:, q * 512:(q + 1) * 512], sB,
                                        EXP, scale=SCALE,
                                        accum_out=sums[:, 4 + it:5 + it])
                            e_t[(0, half)] = eA
                            e_t[(1, half)] = eB
                        # S.T via swapped matmuls + exp -> E.T (unnormalized)
                        et_t = {}
                        for half in range(2):
                            etA = work.tile([128, 1024], BF16, tag="et", bufs=6)
                            etB = work.tile([128, 1024], BF16, tag="et", bufs=6)
                            for q in range(2):
                                jt = 2 * half + q
                                stA = ps_s.tile([128, 512], F32, tag="ps")
                                stB = ps_s.tile([128, 512], F32, tag="ps")
                                nc.tensor.matmul(
                                    stA, kt[hp][0:64, jt * 128:(jt + 1) * 128],
                                    qt[hp][0:64, :], start=True, stop=True,
                                    tile_position=(0, 0))
                                nc.tensor.matmul(
                                    stB, kt[hp][64:128, jt * 128:(jt + 1) * 128],
                                    qt[hp][64:128, :], start=True, stop=True,
                                    tile_position=(64, 0))
                                nc.scalar.activation(
                                    etA[:, q * 512:(q + 1) * 512], stA,
                                    EXP, scale=SCALE)
                                nc.scalar.activation(
                                    etB[:, q * 512:(q + 1) * 512], stB,
                                    EXP, scale=SCALE)
                            et_t[(0, half)] = etA
                            et_t[(1, half)] = etB
                        rec = work.tile([128, 8], F32, tag="rec", bufs=3)
                        nc.vector.reciprocal(rec, sums)

                        # probs output: P = E * (1/s) (bf16), SWDGE cast to f32
                        for head in range(2):
                            p_t = work.tile([128, 4, 512], BF16, tag="p", bufs=3)
                            for it in range(4):
                                nc.vector.tensor_scalar_mul(
                                    p_t[:, it, :],
                                    e_t[(head, it // 2)][:, (it % 2) * 512:
                                                         (it % 2 + 1) * 512],
                                    rec[:, head * 4 + it:head * 4 + it + 1])
                            bh = b * H + hp * 2 + head
                            if "probs_dma" not in ablate:
                                nc.gpsimd.dma_start(
                                    out=probs_ext[bh].rearrange(
                                        "(it p) j -> p it j", p=128),
                                    in_=p_t)

                        # reciprocal rows [1, 512] per head via PE transpose
                        rrows = []
                        for head in range(2):
                            srp = ps_r.tile([1, 512], F32, tag="sr")
                            for it in range(4):
                                nc.tensor.transpose(
                                    srp[0:1, it * 128:(it + 1) * 128],
                                    rec[:, head * 4 + it:head * 4 + it + 1],
                                    identf32)
                            rrow = work.tile([1, 512], BF16, tag="rrow", bufs=4)
                            nc.vector.tensor_copy(rrow, srp)
                            rrows.append(rrow)
                        # recip broadcast [128(c 2-head), 512(i)] via rank-1
                        rbcp = ps_m.tile([128, N], F32, tag="pm")
                        for head in range(2):
                            for it in range(4):
                                nc.tensor.matmul(
                                    rbcp[head * 64:(head + 1) * 64,
                                         it * 128:(it + 1) * 128],
                                    ones[0:1, 0:64],
                                    rrows[head][0:1, it * 128:(it + 1) * 128],
                                    start=True, stop=True)
                        rbc = work.tile([128, N], BF16, tag="rbc", bufs=3)
                        nc.vector.tensor_copy(rbc, rbcp)

                        pctx = ps_m.tile([128, N], F32, tag="pm")
                        for jt in range(NT):
                            nc.tensor.matmul(
                                pctx[0:64, :],
                                vv[jt][:, (2 * hp) * 64:(2 * hp + 1) * 64],
                                et_t[(0, jt // 2)][:, (jt % 2) * 512:
                                                   (jt % 2 + 1) * 512],
                                start=(jt == 0), stop=(jt == NT - 1),
                                tile_position=(0, 0))
                            nc.tensor.matmul(
                                pctx[64:128, :],
                                vv[jt][:, (2 * hp + 1) * 64:(2 * hp + 2) * 64],
                                et_t[(1, jt // 2)][:, (jt % 2) * 512:
                                                   (jt % 2 + 1) * 512],
                                start=(jt == 0), stop=(jt == NT - 1),
                                tile_position=(0, 64))
                        ctt = work.tile([128, N], BF16, tag="ctxt", bufs=9)
                        nc.vector.tensor_mul(ctt, pctx, rbc)
                        ctxt.append(ctt)

                    # ---- out projection
                    for mt in range(NT):
                        po = ps_m.tile([128, N], F32, tag="pm")
                        nc.tensor.matmul(po, ones, bo_row,
                                         start=True, stop=False)
                        for ct in range(CT):
                            nc.tensor.matmul(
                                po, ctxt[ct][:, mt * 128:(mt + 1) * 128],
                                wot[ct], start=False, stop=(ct == CT - 1))
                        ott = work.tile([128, N], F32, tag="outt", bufs=3)
                        nc.vector.tensor_copy(ott, po)
                        nc.sync.dma_start(
                            out=out_ext[b, mt * 128:(mt + 1) * 128, :], in_=ott)

            if repeats == 1:
                body(0)
            else:
                with tc.For_i(0, repeats, 1) as iv:
                    body(iv)

    nc.compile()
    return nc


_CACHED = {}


def _get_nc():
    if "nc" not in _CACHED:
        _CACHED["nc"] = build_attention_nc()
    return _CACHED["nc"]


def kernel(**inputs):
    from concourse.bass_utils import run_bass_kernel_spmd

    nc = _get_nc()
    x = np.ascontiguousarray(np.asarray(inputs["x"], dtype=np.float32))
    shared = {k: np.ascontiguousarray(np.asarray(inputs[k], dtype=np.float32))
              for k in ("Wq", "bq", "Wk", "bk", "Wv", "bv", "Wo", "bo")}
    in_maps = [dict(x=x[c * BS:(c + 1) * BS], **shared) for c in range(NCORES)]
    res = run_bass_kernel_spmd(nc, in_maps, core_ids=list(range(NCORES)))
    out = np.concatenate([res.results[c]["out"] for c in range(NCORES)], axis=0)
    probs = np.concatenate([res.results[c]["probs"] for c in range(NCORES)],
                           axis=0)
    return out, probs
